# revision 1
# baseline (speedup 1.0000x reference)
"""CrossModalAttention Trainium2 kernel.

Data-parallel over batch B=8 across the 8 NeuronCores (core b owns batch b,
weights replicated, no collectives). Within a core all 9 modality-pair
attentions run with bf16 matmuls / fp32 PSUM accumulation.

Layout strategy (per core, batch b fixed):
  xT[m]  : [c, n]  (c on partitions)  -- host pre-transposed, bf16
  qT/kT[m]: [d, n] = Wq[m].T-projection output, d on partitions
  v[m]   : [n_k, d] natural layout, with an extra per-head "ones" column so
           the PV matmul produces the softmax denominator Z in column 64.
  S^T    : [k, n] per (i,j,head) from lhsT=kT-slice, rhs=qT-slice (K=64)
  E = exp(S^T): ACT engine for most tasks; a tunable subset runs on DVE via
           a Schraudolph-style bit-trick (x*C+D -> int16 -> bitcast bf16),
           splitting the exp wall between both engines.
  PV     : out[n-sub, 65] with lhsT=E-slice, rhs=v-slice(65 cols) accumulated
           over key tiles; col 64 = Z * ones_val.
  consume: fused[n, c] += PV[:, 0:64] * reciprocal(Z*ones_val)  (DVE)
  final  : PE-transpose fused -> fusedT, out = fusedT.T @ Wp + bp, split in
           two contraction halves so most of it overlaps the attention tail.

Exact simplifications vs the reference:
  - bk drops entirely: q.(k+bk) adds a per-query constant across keys, which
    softmax over keys cancels.
  - bv folds into bp on the host: sum_k probs = 1, so the v-bias contributes
    (1/M) sum_ij mw_ij bv_j to every fused row; push through Wp into bp.
"""

import sys

import numpy as np

for _p in ("/opt/trn_rl_repo",):
    if _p not in sys.path:
        sys.path.insert(0, _p)

import ml_dtypes  # noqa: E402

import concourse.bass as bass  # noqa: E402
from concourse import bacc  # noqa: E402
import concourse.mybir as mybir  # noqa: E402
import concourse.tile as tile  # noqa: E402

M, B, N, C, H = 3, 8, 512, 512, 8
HD = C // H  # 64
P = 128
CT = C // P  # 4 contraction tiles
NT = N // P  # 4 row tiles
DT = C // P  # 4 output-channel tiles
SCALE = float(HD) ** -0.5

BF16 = mybir.dt.bfloat16
F32 = mybir.dt.float32
I16 = mybir.dt.int16
NP_BF16 = ml_dtypes.bfloat16

AluOp = mybir.AluOpType
ActFn = mybir.ActivationFunctionType

# Schraudolph bf16 exp: exp(x) ~= bitcast_bf16(int16(x*SCH_C + SCH_D))
SCH_SHIFT = 0.0579
SCH_C = 128.0 / float(np.log(2.0))
SCH_D = (127.0 - SCH_SHIFT) * 128.0 + 0.5  # +0.5: assume trunc-to-int store

# tasks whose exp runs on DVE instead of ACT (balance the two engines).
# Kept away from the endgame (DVE owns the consume tail) and never adjacent:
# each DVE task is emitted paired with its successor so ACT keeps lookahead.
DVE_EXP_TASKS = frozenset(range(10, 64, 5))


def _build_bass(mw, uniform, ones_val, reps=1, skip=()):
    """Emit the single-core SPMD program. mw is the [M,M] modal weight matrix
    (values are baked into the program as immediates)."""
    from concourse.masks import make_identity

    nc = bacc.Bacc(None)

    xt_d = nc.dram_tensor("xt", [M, P, CT, N], BF16, kind="ExternalInput")
    wq_d = nc.dram_tensor("wq", [M, P, CT, C], BF16, kind="ExternalInput")
    wk_d = nc.dram_tensor("wk", [M, P, CT, C], BF16, kind="ExternalInput")
    wv_d = nc.dram_tensor("wv", [M, P, CT, C], BF16, kind="ExternalInput")
    wp_d = nc.dram_tensor("wp", [P, CT, C], BF16, kind="ExternalInput")
    bq_d = nc.dram_tensor("bq", [P, M, DT], F32, kind="ExternalInput")
    bp_d = nc.dram_tensor("bp", [1, C], BF16, kind="ExternalInput")
    out_d = nc.dram_tensor("out", [N, C], F32, kind="ExternalOutput")

    with tile.TileContext(nc) as tc:
        with (
            tc.tile_pool(name="consts", bufs=1) as consts,
            tc.tile_pool(name="esb", bufs=3) as esb,
            tc.tile_pool(name="zr", bufs=8) as zrp,
            tc.tile_pool(name="spsum", bufs=3, space="PSUM") as spsum,
            tc.tile_pool(name="pvpsum", bufs=2, space="PSUM") as pvpsum,
        ):
            # ---- persistent SBUF tiles ----
            xt_sb = [consts.tile([P, CT, N], BF16, tag=f"xt{m}", name=f"xt{m}") for m in range(M)]
            wq_sb = [consts.tile([P, CT, C], BF16, tag=f"wq{m}", name=f"wq{m}") for m in range(M)]
            wk_sb = [consts.tile([P, CT, C], BF16, tag=f"wk{m}", name=f"wk{m}") for m in range(M)]
            wv_sb = [consts.tile([P, CT, C], BF16, tag=f"wv{m}", name=f"wv{m}") for m in range(M)]
            wp_sb = consts.tile([P, CT, C], BF16, tag="wp", name="wp")
            bq_sb = consts.tile([P, M, DT], F32, tag="bq", name="bq")
            bp_sb = consts.tile([1, C], BF16, tag="bp", name="bp")
            qT_sb = [consts.tile([P, DT, N], BF16, tag=f"qT{m}", name=f"qT{m}") for m in range(M)]
            kT_sb = [consts.tile([P, DT, N], BF16, tag=f"kT{m}", name=f"kT{m}") for m in range(M)]
            # v with per-head ones column (col 64) for the softmax denominator
            v_sb = [consts.tile([P, NT, H, HD + 1], BF16, tag=f"v{m}", name=f"v{m}") for m in range(M)]
            fused_sb = consts.tile([P, NT, C], F32, tag="fused", name="fused")
            fusedT_sb = consts.tile([P, CT, N], BF16, tag="fusedT", name="fusedT")
            out_sb = consts.tile([P, NT, C], F32, tag="outsb", name="outsb")
            ones_sb = consts.tile([1, P], BF16, tag="ones", name="ones")
            ident_sb = consts.tile([P, P], F32, tag="ident", name="ident")

            # ---- input DMAs: whole-tensor transfers (HWDGE dispatch is
            # ~625ns each, so fewer+bigger beats per-ct chunks), m=0 first ----
            nc.sync.dma_start(out=xt_sb[0], in_=xt_d[0])
            nc.sync.dma_start(out=wq_sb[0], in_=wq_d[0])
            nc.sync.dma_start(out=wk_sb[0], in_=wk_d[0])
            nc.sync.dma_start(out=bq_sb, in_=bq_d[:])
            nc.sync.dma_start(out=wv_sb[0], in_=wv_d[0])
            for m in (1, 2):
                nc.sync.dma_start(out=xt_sb[m], in_=xt_d[m])
                nc.sync.dma_start(out=wk_sb[m], in_=wk_d[m])
                nc.sync.dma_start(out=wv_sb[m], in_=wv_d[m])
                nc.sync.dma_start(out=wq_sb[m], in_=wq_d[m])
            nc.sync.dma_start(out=wp_sb, in_=wp_d[:])
            nc.sync.dma_start(out=bp_sb, in_=bp_d[:])

            nc.gpsimd.memset(ones_sb, 1.0)
            make_identity(nc, ident_sb)
            act_warm = consts.tile([1, 2], F32, tag="actwarm", name="actwarm")
            nc.scalar.activation(act_warm, ones_sb[0:1, 0:2], ActFn.Exp)

            # PE p-state warm-up: dummy matmuls with no DMA deps keep the
            # tensor engine continuously busy (ramping to max clock) while
            # the first input DMAs land.
            warm_sb = consts.tile([P, N], BF16, tag="warm", name="warm")
            nc.gpsimd.memset(warm_sb, 0.0)
            wps = spsum.tile([P, N], F32, tag="s", name="s")
            for _ in range(10):
                nc.tensor.matmul(wps, warm_sb[:, 0:P], warm_sb, start=True, stop=True)

            def emit_qk_proj_dt(m, dt):
                # q (bias via DVE) and k (no bias: softmax-invariant) for one dt
                ps = spsum.tile([P, N], F32, tag="s", name="s")
                for ct in range(CT):
                    nc.tensor.matmul(
                        ps,
                        wq_sb[m][:, ct, dt * P : (dt + 1) * P],
                        xt_sb[m][:, ct, :],
                        start=(ct == 0),
                        stop=(ct == CT - 1),
                    )
                nc.vector.tensor_tensor(
                    qT_sb[m][:, dt, :],
                    ps,
                    bq_sb[:, m, dt : dt + 1].to_broadcast((P, N)),
                    AluOp.add,
                )
                ps = spsum.tile([P, N], F32, tag="s", name="s")
                for ct in range(CT):
                    nc.tensor.matmul(
                        ps,
                        wk_sb[m][:, ct, dt * P : (dt + 1) * P],
                        xt_sb[m][:, ct, :],
                        start=(ct == 0),
                        stop=(ct == CT - 1),
                    )
                nc.vector.tensor_copy(out=kT_sb[m][:, dt, :], in_=ps)

            def emit_v_proj(m, nts):
                # v projection -> natural layout [n_k, d]; bias folded into bp
                for nt in nts:
                    ps = spsum.tile([P, C], F32, tag="s", name="s")
                    for ct in range(CT):
                        nc.tensor.matmul(
                            ps,
                            xt_sb[m][:, ct, nt * P : (nt + 1) * P],
                            wv_sb[m][:, ct, :],
                            start=(ct == 0),
                            stop=(ct == CT - 1),
                        )
                    nc.vector.tensor_copy(
                        out=v_sb[m][:, nt, :, 0:HD],
                        in_=ps.rearrange("p (h e) -> p h e", e=HD),
                    )
                    nc.gpsimd.memset(v_sb[m][:, nt, :, HD : HD + 1], ones_val)

            def emit_qk_chunk(i, j, h, half, e_t, use_dve):
                hof = (h % 2) * HD
                ht = h // 2
                st = spsum.tile([P, 2, N], F32, tag="s", name="s")
                for k2 in range(2):
                    kt = half * 2 + k2
                    nc.tensor.matmul(
                        st[:, k2, :],
                        kT_sb[j][hof : hof + HD, ht, kt * P : (kt + 1) * P],
                        qT_sb[i][hof : hof + HD, ht, :],
                        start=True,
                        stop=True,
                    )
                dst = e_t[:, half * 2 : half * 2 + 2, :]
                if use_dve:
                    nc.vector.tensor_scalar(
                        dst.bitcast(I16),
                        st,
                        SCH_C,
                        SCH_D,
                        AluOp.mult,
                        AluOp.add,
                    )
                else:
                    nc.scalar.activation(dst, st, ActFn.Exp)

            def emit_qk_exp(i, j, h, use_dve):
                e_t = esb.tile([P, CT, N], BF16, tag="E", name="E")
                for half in range(2):
                    emit_qk_chunk(i, j, h, half, e_t, use_dve)
                return e_t

            def emit_pv_consume(i, j, h, e_t, first):
                w_ij = float(mw[i, j])
                pv_t = pvpsum.tile([P, NT, HD + 1], F32, tag="pv", name="pv")
                for ns in range(NT):
                    for kt in range(CT):
                        nc.tensor.matmul(
                            pv_t[:, ns, :],
                            e_t[:, kt, ns * P : (ns + 1) * P],
                            v_sb[j][:, kt, h, :],
                            start=(kt == 0),
                            stop=(kt == CT - 1),
                        )
                # consume: fused[:, :, h] += pv[:, :, 0:64] / (Z * ones_val)
                zr = zrp.tile([P, NT, 1], F32, tag="zr", name="zr")
                nc.vector.reciprocal(zr, pv_t[:, :, HD : HD + 1])
                if not uniform:
                    nc.vector.tensor_scalar(
                        zr, zr, w_ij / float(M), None, AluOp.mult
                    )
                dst = fused_sb[:, :, h * HD : (h + 1) * HD]
                if first:
                    nc.vector.tensor_tensor(
                        dst,
                        pv_t[:, :, 0:HD],
                        zr.to_broadcast((P, NT, HD)),
                        AluOp.mult,
                    )
                else:
                    tmp = zrp.tile([P, NT, HD], F32, tag="ctmp", name="ctmp")
                    nc.vector.tensor_tensor(
                        tmp,
                        pv_t[:, :, 0:HD],
                        zr.to_broadcast((P, NT, HD)),
                        AluOp.mult,
                    )
                    nc.vector.tensor_tensor(dst, dst, tmp, AluOp.add)

            def emit_transpose_nt(ct, nt):
                tpf = spsum.tile([P, C], F32, tag="s", name="s")
                tp = tpf[:, 0:P]
                nc.tensor.transpose(
                    tp, fused_sb[:, nt, ct * P : (ct + 1) * P], ident_sb
                )
                dst = fusedT_sb[:, ct, nt * P : (nt + 1) * P]
                if ct == CT - 1:
                    # endgame: ACT is idle by now, DVE owns the consume tail
                    nc.scalar.copy(out=dst, in_=tp)
                else:
                    nc.vector.tensor_copy(out=dst, in_=tp)

            def emit_final_a_nt(nt):
                # first contraction half (ct 0,1) + bias; overlaps attention
                ps = spsum.tile([P, C], F32, tag="s", name="s")
                for ct in (0, 1):
                    nc.tensor.matmul(
                        ps,
                        fusedT_sb[:, ct, nt * P : (nt + 1) * P],
                        wp_sb[:, ct, :],
                        start=(ct == 0),
                        stop=False,
                    )
                nc.tensor.matmul(
                    ps, ones_sb[0:1, :], bp_sb[0:1, :], start=False, stop=True
                )
                nc.vector.tensor_copy(out=out_sb[:, nt, :], in_=ps)

            def emit_final_b():
                for nt in range(NT):
                    ps = spsum.tile([P, C], F32, tag="s", name="s")
                    for ct in (2, 3):
                        nc.tensor.matmul(
                            ps,
                            fusedT_sb[:, ct, nt * P : (nt + 1) * P],
                            wp_sb[:, ct, :],
                            start=(ct == 2),
                            stop=(ct == 3),
                        )
                    nc.vector.tensor_tensor(
                        out_sb[:, nt, :], out_sb[:, nt, :], ps, AluOp.add
                    )
                    nc.sync.dma_start(
                        out=out_d[nt * P : (nt + 1) * P, :], in_=out_sb[:, nt, :]
                    )

            # ---- emission schedule ----
            # (0,0) runs all 8 heads first; the rest go head-major so fused
            # head-pair slices complete early and transpose mid-stream.
            rest_pairs = (
                (0, 1), (1, 0), (1, 1), (0, 2), (2, 0), (1, 2), (2, 1), (2, 2),
            )
            tasks = [(0, 0, h) for h in range(H)] + [
                (i, j, h) for h in range(H) for (i, j) in rest_pairs
            ]

            # proj chunks popped one per task emission; order chosen so every
            # task's qT/kT/v dependencies are emitted before first use.
            def chunk_list():
                return [
                    lambda: emit_qk_proj_dt(0, 1),
                    lambda: emit_v_proj(0, (0, 1)),
                    lambda: emit_v_proj(0, (2, 3)),
                    lambda: emit_qk_proj_dt(0, 2),
                    lambda: emit_qk_proj_dt(0, 3),
                    lambda: emit_qk_proj_dt(1, 0),
                    lambda: emit_v_proj(1, (0, 1)),
                    lambda: emit_v_proj(1, (2, 3)),
                    lambda: emit_qk_proj_dt(2, 0),
                    lambda: emit_v_proj(2, (0, 1)),
                    lambda: emit_v_proj(2, (2, 3)),
                    lambda: emit_qk_proj_dt(1, 1),
                    lambda: emit_qk_proj_dt(2, 1),
                    lambda: emit_qk_proj_dt(1, 2),
                    lambda: emit_qk_proj_dt(2, 2),
                    lambda: emit_qk_proj_dt(1, 3),
                    lambda: emit_qk_proj_dt(2, 3),
                ]
            # m/dt available after popping chunk index:
            qk_avail = {(0, 0): -1, (0, 1): 0, (0, 2): 3, (0, 3): 4,
                        (1, 0): 5, (2, 0): 8, (1, 1): 11, (2, 1): 12,
                        (1, 2): 13, (2, 2): 14, (1, 3): 15, (2, 3): 16}
            v_avail = {0: 2, 1: 7, 2: 10}

            for _rep in range(reps):
                emit_qk_proj_dt(0, 0)
                chunks = chunk_list()
                popped = -1  # index of last popped chunk
                consumed = [0] * H
                transposed = [False] * CT
                final_a_done = False
                pending = []
                deferred = []  # staggered transpose/final_a pieces

                def flush_one():
                    nonlocal final_a_done
                    pi, pj, ph, pe = pending.pop(0)
                    emit_pv_consume(pi, pj, ph, pe, first=(pi == 0 and pj == 0))
                    consumed[ph] += 1
                    ct = ph // 2
                    if (
                        not transposed[ct]
                        and consumed[2 * ct] == M * M
                        and consumed[2 * ct + 1] == M * M
                    ):
                        transposed[ct] = True
                        for nt in range(NT):
                            deferred.append(
                                lambda ct=ct, nt=nt: emit_transpose_nt(ct, nt)
                            )
                        if not final_a_done and transposed[0] and transposed[1]:
                            final_a_done = True
                            for nt in range(NT):
                                deferred.append(
                                    lambda nt=nt: emit_final_a_nt(nt)
                                )
                    if deferred:
                        deferred.pop(0)()

                n = 0
                while n < len(tasks):
                    # a DVE-exp task is emitted together with its successor so
                    # ACT gets its next chunk without a full task-latency gap
                    group = [n]
                    if n in DVE_EXP_TASKS and n + 1 < len(tasks):
                        group.append(n + 1)
                    for g in group:
                        i, j, h = tasks[g]
                        assert qk_avail[(i, h // 2)] <= popped, (g, i, j, h)
                        assert qk_avail[(j, h // 2)] <= popped, (g, i, j, h)
                    if len(group) == 2:
                        # chunk-interleaved: ACT task (n+1) chunks lead so the
                        # ACT engine never waits a full task latency
                        nD, nA = group
                        eD = esb.tile([P, CT, N], BF16, tag="E", name="E")
                        eA = esb.tile([P, CT, N], BF16, tag="E", name="E")
                        for half in range(2):
                            iA, jA, hA = tasks[nA]
                            emit_qk_chunk(iA, jA, hA, half, eA, use_dve=False)
                            iD, jD, hD = tasks[nD]
                            emit_qk_chunk(iD, jD, hD, half, eD, use_dve=True)
                        pending.append((*tasks[nD], eD))
                        pending.append((*tasks[nA], eA))
                    else:
                        i, j, h = tasks[n]
                        e_t = emit_qk_exp(i, j, h, use_dve=False)
                        pending.append((i, j, h, e_t))
                    # one proj chunk per group (two back-to-back would starve
                    # ACT for ~3.4us of PE time)
                    if popped + 1 < len(chunks):
                        popped += 1
                        chunks[popped]()
                    while len(pending) > 1 and v_avail[pending[0][1]] <= popped:
                        flush_one()
                    assert len(pending) <= 3, n  # esb pool depth
                    n = group[-1] + 1
                while pending:
                    assert v_avail[pending[0][1]] <= popped
                    flush_one()
                assert all(transposed) and final_a_done
                while deferred:
                    deferred.pop(0)()
                emit_final_b()

    nc.compile()
    return nc


def _prep_inputs(x, Wq, bq, Wk, bk, Wv, bv, mw, Wp, bp):
    """Host-side shard + retile. Returns (in_maps, uniform, ones_val, mw)."""
    x = np.asarray(x, dtype=np.float32)
    Wq = np.asarray(Wq, dtype=np.float32)
    bq = np.asarray(bq, dtype=np.float32)
    Wk = np.asarray(Wk, dtype=np.float32)
    Wv = np.asarray(Wv, dtype=np.float32)
    bv = np.asarray(bv, dtype=np.float64)
    mw = np.asarray(mw, dtype=np.float64)
    Wp = np.asarray(Wp, dtype=np.float64)
    bp = np.asarray(bp, dtype=np.float64)

    w0 = float(mw.flat[0])
    uniform = bool(np.all(mw == w0)) and abs(w0) > 1e-6
    if uniform:
        ones_val = float(NP_BF16(M / w0))
        # compensate bf16 rounding of ones_val exactly through Wp
        kappa = ones_val * w0 / M
    else:
        ones_val = 1.0
        kappa = 1.0

    # fold v-bias through Wp into bp: sum_k probs = 1 exactly, so each pair
    # contributes (mw_ij/M) * bv_j to every fused row.
    bias_v = (mw[:, :, None] * bv[None, :, :]).sum(axis=(0, 1)) / M  # [C]
    bp_eff = bp + bias_v @ Wp

    def tile_w(w):  # [C, C] -> [P, CT, C]
        return np.ascontiguousarray(
            w.reshape(CT, P, C).transpose(1, 0, 2).astype(NP_BF16)
        )

    wq_h = np.stack([tile_w(Wq[m] * SCALE) for m in range(M)])
    wk_h = np.stack([tile_w(Wk[m]) for m in range(M)])
    wv_h = np.stack([tile_w(Wv[m]) for m in range(M)])
    wp_h = tile_w((Wp / kappa).astype(np.float32))

    def tile_b(b):  # [C] -> [P, DT]
        return np.ascontiguousarray(b.reshape(DT, P).T.astype(np.float32))

    bq_h = np.ascontiguousarray(
        np.stack([tile_b(bq[m] * SCALE) for m in range(M)]).transpose(1, 0, 2)
    )
    bp_h = np.ascontiguousarray(bp_eff.reshape(1, C).astype(NP_BF16))

    # x [M,B,N,C] -> per-core xT [M,P,CT,N]
    xt_all = np.ascontiguousarray(
        x.transpose(1, 0, 3, 2)  # [B, M, C, N]
        .reshape(B, M, CT, P, N)
        .transpose(0, 1, 3, 2, 4)  # [B, M, P, CT, N]
        .astype(NP_BF16)
    )

    common = {
        "wq": wq_h,
        "wk": wk_h,
        "wv": wv_h,
        "wp": wp_h,
        "bq": bq_h,
        "bp": bp_h,
    }
    in_maps = [dict(common, xt=np.ascontiguousarray(xt_all[b])) for b in range(B)]
    return in_maps, uniform, ones_val, mw


def run(trace=False, **inputs):
    from concourse.bass_utils import run_bass_kernel_spmd

    in_maps, uniform, ones_val, mw = _prep_inputs(**inputs)
    nc = _build_bass(mw, uniform, ones_val)
    res = run_bass_kernel_spmd(
        nc, in_maps, core_ids=list(range(B)), trace=trace
    )
    out = np.stack([res.results[b]["out"] for b in range(B)]).astype(np.float32)
    return out, res


def kernel(**inputs):
    out, _ = run(trace=False, **inputs)
    return out


if __name__ == "__main__":
    rng = np.random.default_rng(0)
    ins = {
        "x": rng.standard_normal((M, B, N, C), dtype=np.float32),
        "Wq": rng.standard_normal((M, C, C), dtype=np.float32) * 0.02,
        "bq": rng.standard_normal((M, C), dtype=np.float32) * 0.02,
        "Wk": rng.standard_normal((M, C, C), dtype=np.float32) * 0.02,
        "bk": rng.standard_normal((M, C), dtype=np.float32) * 0.02,
        "Wv": rng.standard_normal((M, C, C), dtype=np.float32) * 0.02,
        "bv": rng.standard_normal((M, C), dtype=np.float32) * 0.02,
        "mw": np.ones((M, M), dtype=np.float32),
        "Wp": rng.standard_normal((C, C), dtype=np.float32) * 0.02,
        "bp": rng.standard_normal((C,), dtype=np.float32) * 0.02,
    }
    out = kernel(**ins)
    print("out", out.shape, out.dtype, float(np.abs(out).mean()))



# revision 17
# speedup vs baseline: 1.0064x; 1.0064x over previous
"""CrossModalAttention Trainium2 kernel.

Data-parallel over batch B=8 across the 8 NeuronCores (core b owns batch b,
weights replicated, no collectives). Within a core all 9 modality-pair
attentions run with bf16 matmuls / fp32 PSUM accumulation.

Layout strategy (per core, batch b fixed):
  xT[m]  : [c, n]  (c on partitions)  -- host pre-transposed, bf16
  qT/kT[m]: [d, n] = Wq[m].T-projection output, d on partitions
  v[m]   : [n_k, d] natural layout, with an extra per-head "ones" column so
           the PV matmul produces the softmax denominator Z in column 64.
  S^T    : [k, n] per (i,j,head) from lhsT=kT-slice, rhs=qT-slice (K=64)
  E = exp(S^T): ACT engine for most tasks; a tunable subset runs on DVE via
           a Schraudolph-style bit-trick (x*C+D -> int16 -> bitcast bf16),
           splitting the exp wall between both engines.
  PV     : out[n-sub, 65] with lhsT=E-slice, rhs=v-slice(65 cols) accumulated
           over key tiles; col 64 = Z * ones_val.
  consume: fused[n, c] += PV[:, 0:64] * reciprocal(Z*ones_val)  (DVE)
  final  : PE-transpose fused -> fusedT, out = fusedT.T @ Wp + bp, split in
           two contraction halves so most of it overlaps the attention tail.

Exact simplifications vs the reference:
  - bk drops entirely: q.(k+bk) adds a per-query constant across keys, which
    softmax over keys cancels.
  - bv folds into bp on the host: sum_k probs = 1, so the v-bias contributes
    (1/M) sum_ij mw_ij bv_j to every fused row; push through Wp into bp.
"""

import sys

import numpy as np

for _p in ("/opt/trn_rl_repo",):
    if _p not in sys.path:
        sys.path.insert(0, _p)

import ml_dtypes  # noqa: E402

import concourse.bass as bass  # noqa: E402
from concourse import bacc  # noqa: E402
import concourse.mybir as mybir  # noqa: E402
import concourse.tile as tile  # noqa: E402

M, B, N, C, H = 3, 8, 512, 512, 8
HD = C // H  # 64
P = 128
CT = C // P  # 4 contraction tiles
NT = N // P  # 4 row tiles
DT = C // P  # 4 output-channel tiles
SCALE = float(HD) ** -0.5

BF16 = mybir.dt.bfloat16
F32 = mybir.dt.float32
I16 = mybir.dt.int16
NP_BF16 = ml_dtypes.bfloat16

AluOp = mybir.AluOpType
ActFn = mybir.ActivationFunctionType

# Schraudolph bf16 exp: exp(x) ~= bitcast_bf16(int16(x*SCH_C + SCH_D))
SCH_SHIFT = 0.0579
SCH_C = 128.0 / float(np.log(2.0))
SCH_D = (127.0 - SCH_SHIFT) * 128.0 + 0.5  # +0.5: assume trunc-to-int store

# tasks whose exp runs on DVE instead of ACT (balance the two engines).
# Consume runs on Pool now, so DVE's only other load is the proj copies
# (early) — weight DVE-exp toward the mid/late stream. Never adjacent:
# each DVE task is emitted paired with its successor so ACT keeps lookahead.
DVE_EXP_TASKS = frozenset(range(10, 68, 3))


def _build_bass(mw, uniform, ones_val, reps=1, skip=()):
    """Emit the single-core SPMD program. mw is the [M,M] modal weight matrix
    (values are baked into the program as immediates)."""
    from concourse.masks import make_identity

    nc = bacc.Bacc(None)

    xt_d = nc.dram_tensor("xt", [M, P, CT, N], BF16, kind="ExternalInput")
    wq_d = nc.dram_tensor("wq", [M, P, CT, C], BF16, kind="ExternalInput")
    wk_d = nc.dram_tensor("wk", [M, P, CT, C], BF16, kind="ExternalInput")
    wv_d = nc.dram_tensor("wv", [M, P, CT, C], BF16, kind="ExternalInput")
    wp_d = nc.dram_tensor("wp", [P, CT, C], BF16, kind="ExternalInput")
    bq_d = nc.dram_tensor("bq", [P, M, DT], F32, kind="ExternalInput")
    bp_d = nc.dram_tensor("bp", [1, C], BF16, kind="ExternalInput")
    out_d = nc.dram_tensor("out", [N, C], F32, kind="ExternalOutput")

    with tile.TileContext(nc) as tc:
        with (
            tc.tile_pool(name="consts", bufs=1) as consts,
            tc.tile_pool(name="esb", bufs=3) as esb,
            tc.tile_pool(name="zr", bufs=8) as zrp,
            tc.tile_pool(name="spsum", bufs=3, space="PSUM") as spsum,
            tc.tile_pool(name="pvpsum", bufs=2, space="PSUM") as pvpsum,
        ):
            # ---- persistent SBUF tiles ----
            xt_sb = [consts.tile([P, CT, N], BF16, tag=f"xt{m}", name=f"xt{m}") for m in range(M)]
            wq_sb = [consts.tile([P, CT, C], BF16, tag=f"wq{m}", name=f"wq{m}") for m in range(M)]
            wk_sb = [consts.tile([P, CT, C], BF16, tag=f"wk{m}", name=f"wk{m}") for m in range(M)]
            wv_sb = [consts.tile([P, CT, C], BF16, tag=f"wv{m}", name=f"wv{m}") for m in range(M)]
            wp_sb = consts.tile([P, CT, C], BF16, tag="wp", name="wp")
            bq_sb = consts.tile([P, M, DT], F32, tag="bq", name="bq")
            bp_sb = consts.tile([1, C], BF16, tag="bp", name="bp")
            qT_sb = [consts.tile([P, DT, N], BF16, tag=f"qT{m}", name=f"qT{m}") for m in range(M)]
            kT_sb = [consts.tile([P, DT, N], BF16, tag=f"kT{m}", name=f"kT{m}") for m in range(M)]
            # v with per-head ones column (col 64) for the softmax denominator
            v_sb = [consts.tile([P, NT, H, HD + 1], BF16, tag=f"v{m}", name=f"v{m}") for m in range(M)]
            fused_sb = consts.tile([P, NT, C], F32, tag="fused", name="fused")
            fusedT_sb = consts.tile([P, CT, N], BF16, tag="fusedT", name="fusedT")
            out_sb = consts.tile([P, NT, C], F32, tag="outsb", name="outsb")
            ones_sb = consts.tile([1, P], BF16, tag="ones", name="ones")
            ident_sb = consts.tile([P, P], F32, tag="ident", name="ident")

            # ---- input DMAs: whole-tensor transfers (HWDGE dispatch is
            # ~625ns each and serial per queue, so spread dispatch across the
            # SP, ACT and DVE queues; m=0 first, DMA-complete sem adds ~900ns)
            nc.sync.dma_start(out=bq_sb, in_=bq_d[:])
            nc.sync.dma_start(out=xt_sb[0], in_=xt_d[0])
            nc.sync.dma_start(out=wq_sb[0], in_=wq_d[0])
            nc.sync.dma_start(out=wk_sb[0], in_=wk_d[0])
            nc.sync.dma_start(out=wv_sb[0], in_=wv_d[0])
            for m in (1, 2):
                nc.sync.dma_start(out=xt_sb[m], in_=xt_d[m])
                nc.sync.dma_start(out=wk_sb[m], in_=wk_d[m])
                nc.sync.dma_start(out=wv_sb[m], in_=wv_d[m])
                nc.sync.dma_start(out=wq_sb[m], in_=wq_d[m])
            nc.sync.dma_start(out=wp_sb, in_=wp_d[:])
            nc.sync.dma_start(out=bp_sb, in_=bp_d[:])

            nc.gpsimd.memset(ones_sb, 1.0)
            make_identity(nc, ident_sb)
            act_warm = consts.tile([1, 2], F32, tag="actwarm", name="actwarm")
            nc.scalar.activation(act_warm, ones_sb[0:1, 0:2], ActFn.Exp)

            # PE p-state warm-up: dummy matmuls with no DMA deps keep the
            # tensor engine continuously busy (ramping to max clock) while
            # the first input DMAs land.
            warm_sb = consts.tile([P, N], BF16, tag="warm", name="warm")
            nc.vector.memset(warm_sb, 0.0)
            wps = spsum.tile([P, N], F32, tag="s", name="s")
            for _ in range(7):
                nc.tensor.matmul(wps, warm_sb[:, 0:P], warm_sb, start=True, stop=True)

            def emit_qk_proj_dt(m, dt):
                # q (bias via DVE) and k (no bias: softmax-invariant) for one dt
                ps = spsum.tile([P, N], F32, tag="s", name="s")
                for ct in range(CT):
                    nc.tensor.matmul(
                        ps,
                        wq_sb[m][:, ct, dt * P : (dt + 1) * P],
                        xt_sb[m][:, ct, :],
                        start=(ct == 0),
                        stop=(ct == CT - 1),
                    )
                nc.vector.tensor_tensor(
                    qT_sb[m][:, dt, :],
                    ps,
                    bq_sb[:, m, dt : dt + 1].to_broadcast((P, N)),
                    AluOp.add,
                )
                ps = spsum.tile([P, N], F32, tag="s", name="s")
                for ct in range(CT):
                    nc.tensor.matmul(
                        ps,
                        wk_sb[m][:, ct, dt * P : (dt + 1) * P],
                        xt_sb[m][:, ct, :],
                        start=(ct == 0),
                        stop=(ct == CT - 1),
                    )
                nc.vector.tensor_copy(out=kT_sb[m][:, dt, :], in_=ps)

            def emit_v_proj(m, nts):
                # v projection -> natural layout [n_k, d]; bias folded into bp
                for nt in nts:
                    ps = spsum.tile([P, C], F32, tag="s", name="s")
                    for ct in range(CT):
                        nc.tensor.matmul(
                            ps,
                            xt_sb[m][:, ct, nt * P : (nt + 1) * P],
                            wv_sb[m][:, ct, :],
                            start=(ct == 0),
                            stop=(ct == CT - 1),
                        )
                    nc.vector.tensor_copy(
                        out=v_sb[m][:, nt, :, 0:HD],
                        in_=ps.rearrange("p (h e) -> p h e", e=HD),
                    )
                    nc.gpsimd.memset(v_sb[m][:, nt, :, HD : HD + 1], ones_val)

            def emit_qk_chunk(i, j, h, half, e_t, use_dve):
                hof = (h % 2) * HD
                ht = h // 2
                st = spsum.tile([P, 2, N], F32, tag="s", name="s")
                for k2 in range(2):
                    kt = half * 2 + k2
                    nc.tensor.matmul(
                        st[:, k2, :],
                        kT_sb[j][hof : hof + HD, ht, kt * P : (kt + 1) * P],
                        qT_sb[i][hof : hof + HD, ht, :],
                        start=True,
                        stop=True,
                    )
                dst = e_t[:, half * 2 : half * 2 + 2, :]
                if use_dve:
                    nc.vector.tensor_scalar(
                        dst.bitcast(I16),
                        st,
                        SCH_C,
                        SCH_D,
                        AluOp.mult,
                        AluOp.add,
                    )
                else:
                    nc.scalar.activation(dst, st, ActFn.Exp)

            def emit_qk_exp(i, j, h, use_dve):
                e_t = esb.tile([P, CT, N], BF16, tag="E", name="E")
                for half in range(2):
                    emit_qk_chunk(i, j, h, half, e_t, use_dve)
                return e_t

            def emit_pv_consume(i, j, h, e_t, first):
                w_ij = float(mw[i, j])
                pv_t = pvpsum.tile([P, NT, HD + 1], F32, tag="pv", name="pv")
                for ns in range(NT):
                    for kt in range(CT):
                        nc.tensor.matmul(
                            pv_t[:, ns, :],
                            e_t[:, kt, ns * P : (ns + 1) * P],
                            v_sb[j][:, kt, h, :],
                            start=(kt == 0),
                            stop=(kt == CT - 1),
                        )
                # consume: tmp = pv / (Z * ones_val) on DVE (Pool cannot read
                # PSUM), then fused += tmp on Pool (SBUF-only). The per-pair
                # modal weight is baked into ones_val on the uniform path.
                dst = fused_sb[:, :, h * HD : (h + 1) * HD]
                zr = zrp.tile([P, NT, 1], F32, tag="zr", name="zr")
                nc.vector.reciprocal(zr, pv_t[:, :, HD : HD + 1])
                if not uniform:
                    nc.vector.tensor_scalar(
                        zr, zr, w_ij / float(M), None, AluOp.mult
                    )
                if first:
                    nc.vector.tensor_tensor(
                        dst,
                        pv_t[:, :, 0:HD],
                        zr.to_broadcast((P, NT, HD)),
                        AluOp.mult,
                    )
                else:
                    tmp = zrp.tile([P, NT, HD], F32, tag="ctmp", name="ctmp")
                    nc.vector.tensor_tensor(
                        tmp,
                        pv_t[:, :, 0:HD],
                        zr.to_broadcast((P, NT, HD)),
                        AluOp.mult,
                    )
                    nc.gpsimd.tensor_tensor(dst, dst, tmp, AluOp.add)

            def emit_transpose_nt(ct, nt):
                tpf = spsum.tile([P, C], F32, tag="s", name="s")
                tp = tpf[:, 0:P]
                nc.tensor.transpose(
                    tp, fused_sb[:, nt, ct * P : (ct + 1) * P], ident_sb
                )
                dst = fusedT_sb[:, ct, nt * P : (nt + 1) * P]
                if ct == CT - 1:
                    # endgame: ACT is idle by now, DVE owns the consume tail
                    nc.scalar.copy(out=dst, in_=tp)
                else:
                    nc.vector.tensor_copy(out=dst, in_=tp)

            def emit_final_a_nt(nt):
                # first contraction half (ct 0,1) + bias; overlaps attention
                ps = spsum.tile([P, C], F32, tag="s", name="s")
                for ct in (0, 1):
                    nc.tensor.matmul(
                        ps,
                        fusedT_sb[:, ct, nt * P : (nt + 1) * P],
                        wp_sb[:, ct, :],
                        start=(ct == 0),
                        stop=False,
                    )
                nc.tensor.matmul(
                    ps, ones_sb[0:1, :], bp_sb[0:1, :], start=False, stop=True
                )
                nc.vector.tensor_copy(out=out_sb[:, nt, :], in_=ps)

            def emit_final_b():
                for nt in range(NT):
                    ps = spsum.tile([P, C], F32, tag="s", name="s")
                    for ct in (2, 3):
                        nc.tensor.matmul(
                            ps,
                            fusedT_sb[:, ct, nt * P : (nt + 1) * P],
                            wp_sb[:, ct, :],
                            start=(ct == 2),
                            stop=(ct == 3),
                        )
                    nc.vector.tensor_tensor(
                        out_sb[:, nt, :], out_sb[:, nt, :], ps, AluOp.add
                    )
                    nc.sync.dma_start(
                        out=out_d[nt * P : (nt + 1) * P, :], in_=out_sb[:, nt, :]
                    )

            # ---- emission schedule ----
            # (0,0) runs all 8 heads first; the rest go head-major so fused
            # head-pair slices complete early and transpose mid-stream.
            rest_pairs = (
                (0, 1), (1, 0), (1, 1), (0, 2), (2, 0), (1, 2), (2, 1), (2, 2),
            )
            tasks = [(0, 0, h) for h in range(H)] + [
                (i, j, h) for h in range(H) for (i, j) in rest_pairs
            ]

            # proj chunks popped one per task emission; order chosen so every
            # task's qT/kT/v dependencies are emitted before first use.
            def chunk_list():
                return [
                    lambda: emit_qk_proj_dt(0, 1),
                    lambda: emit_v_proj(0, (0, 1)),
                    lambda: emit_v_proj(0, (2, 3)),
                    lambda: emit_qk_proj_dt(0, 2),
                    lambda: emit_qk_proj_dt(0, 3),
                    lambda: emit_qk_proj_dt(1, 0),
                    lambda: emit_v_proj(1, (0, 1)),
                    lambda: emit_v_proj(1, (2, 3)),
                    lambda: emit_qk_proj_dt(2, 0),
                    lambda: emit_v_proj(2, (0, 1)),
                    lambda: emit_v_proj(2, (2, 3)),
                    lambda: emit_qk_proj_dt(1, 1),
                    lambda: emit_qk_proj_dt(2, 1),
                    lambda: emit_qk_proj_dt(1, 2),
                    lambda: emit_qk_proj_dt(2, 2),
                    lambda: emit_qk_proj_dt(1, 3),
                    lambda: emit_qk_proj_dt(2, 3),
                ]
            # m/dt available after popping chunk index:
            qk_avail = {(0, 0): -1, (0, 1): 0, (0, 2): 3, (0, 3): 4,
                        (1, 0): 5, (2, 0): 8, (1, 1): 11, (2, 1): 12,
                        (1, 2): 13, (2, 2): 14, (1, 3): 15, (2, 3): 16}
            v_avail = {0: 2, 1: 7, 2: 10}

            for _rep in range(reps):
                emit_qk_proj_dt(0, 0)
                chunks = chunk_list()
                popped = -1  # index of last popped chunk
                consumed = [0] * H
                transposed = [False] * CT
                final_a_done = False
                pending = []
                deferred = []  # staggered transpose/final_a pieces

                def flush_one():
                    nonlocal final_a_done
                    pi, pj, ph, pe = pending.pop(0)
                    emit_pv_consume(pi, pj, ph, pe, first=(pi == 0 and pj == 0))
                    consumed[ph] += 1
                    ct = ph // 2
                    if (
                        not transposed[ct]
                        and consumed[2 * ct] == M * M
                        and consumed[2 * ct + 1] == M * M
                    ):
                        transposed[ct] = True
                        for nt in range(NT):
                            deferred.append(
                                lambda ct=ct, nt=nt: emit_transpose_nt(ct, nt)
                            )
                        if not final_a_done and transposed[0] and transposed[1]:
                            final_a_done = True
                            for nt in range(NT):
                                deferred.append(
                                    lambda nt=nt: emit_final_a_nt(nt)
                                )
                    if deferred:
                        deferred.pop(0)()

                n = 0
                while n < len(tasks):
                    # a DVE-exp task is emitted together with its successor so
                    # ACT gets its next chunk without a full task-latency gap
                    group = [n]
                    if n in DVE_EXP_TASKS and n + 1 < len(tasks):
                        group.append(n + 1)
                    for g in group:
                        i, j, h = tasks[g]
                        assert qk_avail[(i, h // 2)] <= popped, (g, i, j, h)
                        assert qk_avail[(j, h // 2)] <= popped, (g, i, j, h)
                    if len(group) == 2:
                        # chunk-interleaved: ACT task (n+1) chunks lead so the
                        # ACT engine never waits a full task latency
                        nD, nA = group
                        eD = esb.tile([P, CT, N], BF16, tag="E", name="E")
                        eA = esb.tile([P, CT, N], BF16, tag="E", name="E")
                        for half in range(2):
                            iA, jA, hA = tasks[nA]
                            emit_qk_chunk(iA, jA, hA, half, eA, use_dve=False)
                            iD, jD, hD = tasks[nD]
                            emit_qk_chunk(iD, jD, hD, half, eD, use_dve=True)
                        pending.append((*tasks[nD], eD))
                        pending.append((*tasks[nA], eA))
                    else:
                        i, j, h = tasks[n]
                        e_t = emit_qk_exp(i, j, h, use_dve=False)
                        pending.append((i, j, h, e_t))
                    # one proj chunk per group (two back-to-back would starve
                    # ACT for ~3.4us of PE time)
                    if popped + 1 < len(chunks):
                        popped += 1
                        chunks[popped]()
                    while len(pending) > 1 and v_avail[pending[0][1]] <= popped:
                        flush_one()
                    assert len(pending) <= 3, n  # esb pool depth
                    n = group[-1] + 1
                while pending:
                    assert v_avail[pending[0][1]] <= popped
                    flush_one()
                assert all(transposed) and final_a_done
                while deferred:
                    deferred.pop(0)()
                emit_final_b()

    nc.compile()
    return nc


def _prep_inputs(x, Wq, bq, Wk, bk, Wv, bv, mw, Wp, bp):
    """Host-side shard + retile. Returns (in_maps, uniform, ones_val, mw)."""
    x = np.asarray(x, dtype=np.float32)
    Wq = np.asarray(Wq, dtype=np.float32)
    bq = np.asarray(bq, dtype=np.float32)
    Wk = np.asarray(Wk, dtype=np.float32)
    Wv = np.asarray(Wv, dtype=np.float32)
    bv = np.asarray(bv, dtype=np.float64)
    mw = np.asarray(mw, dtype=np.float64)
    Wp = np.asarray(Wp, dtype=np.float64)
    bp = np.asarray(bp, dtype=np.float64)

    w0 = float(mw.flat[0])
    uniform = bool(np.all(mw == w0)) and abs(w0) > 1e-6
    if uniform:
        ones_val = float(NP_BF16(M / w0))
        # compensate bf16 rounding of ones_val exactly through Wp
        kappa = ones_val * w0 / M
    else:
        ones_val = 1.0
        kappa = 1.0

    # fold v-bias through Wp into bp: sum_k probs = 1 exactly, so each pair
    # contributes (mw_ij/M) * bv_j to every fused row.
    bias_v = (mw[:, :, None] * bv[None, :, :]).sum(axis=(0, 1)) / M  # [C]
    bp_eff = bp + bias_v @ Wp

    def tile_w(w):  # [C, C] -> [P, CT, C]
        return np.ascontiguousarray(
            w.reshape(CT, P, C).transpose(1, 0, 2).astype(NP_BF16)
        )

    wq_h = np.stack([tile_w(Wq[m] * SCALE) for m in range(M)])
    wk_h = np.stack([tile_w(Wk[m]) for m in range(M)])
    wv_h = np.stack([tile_w(Wv[m]) for m in range(M)])
    wp_h = tile_w((Wp / kappa).astype(np.float32))

    def tile_b(b):  # [C] -> [P, DT]
        return np.ascontiguousarray(b.reshape(DT, P).T.astype(np.float32))

    bq_h = np.ascontiguousarray(
        np.stack([tile_b(bq[m] * SCALE) for m in range(M)]).transpose(1, 0, 2)
    )
    bp_h = np.ascontiguousarray(bp_eff.reshape(1, C).astype(NP_BF16))

    # x [M,B,N,C] -> per-core xT [M,P,CT,N]
    xt_all = np.ascontiguousarray(
        x.transpose(1, 0, 3, 2)  # [B, M, C, N]
        .reshape(B, M, CT, P, N)
        .transpose(0, 1, 3, 2, 4)  # [B, M, P, CT, N]
        .astype(NP_BF16)
    )

    common = {
        "wq": wq_h,
        "wk": wk_h,
        "wv": wv_h,
        "wp": wp_h,
        "bq": bq_h,
        "bp": bp_h,
    }
    in_maps = [dict(common, xt=np.ascontiguousarray(xt_all[b])) for b in range(B)]
    return in_maps, uniform, ones_val, mw


def run(trace=False, **inputs):
    from concourse.bass_utils import run_bass_kernel_spmd

    in_maps, uniform, ones_val, mw = _prep_inputs(**inputs)
    nc = _build_bass(mw, uniform, ones_val)
    res = run_bass_kernel_spmd(
        nc, in_maps, core_ids=list(range(B)), trace=trace
    )
    out = np.stack([res.results[b]["out"] for b in range(B)]).astype(np.float32)
    return out, res


def kernel(**inputs):
    out, _ = run(trace=False, **inputs)
    return out


if __name__ == "__main__":
    rng = np.random.default_rng(0)
    ins = {
        "x": rng.standard_normal((M, B, N, C), dtype=np.float32),
        "Wq": rng.standard_normal((M, C, C), dtype=np.float32) * 0.02,
        "bq": rng.standard_normal((M, C), dtype=np.float32) * 0.02,
        "Wk": rng.standard_normal((M, C, C), dtype=np.float32) * 0.02,
        "bk": rng.standard_normal((M, C), dtype=np.float32) * 0.02,
        "Wv": rng.standard_normal((M, C, C), dtype=np.float32) * 0.02,
        "bv": rng.standard_normal((M, C), dtype=np.float32) * 0.02,
        "mw": np.ones((M, M), dtype=np.float32),
        "Wp": rng.standard_normal((C, C), dtype=np.float32) * 0.02,
        "bp": rng.standard_normal((C,), dtype=np.float32) * 0.02,
    }
    out = kernel(**ins)
    print("out", out.shape, out.dtype, float(np.abs(out).mean()))



# revision 22
# speedup vs baseline: 1.0433x; 1.0366x over previous
"""CrossModalAttention Trainium2 kernel.

Data-parallel over batch B=8 across the 8 NeuronCores (core b owns batch b,
weights replicated, no collectives). Within a core all 9 modality-pair
attentions run with bf16 matmuls / fp32 PSUM accumulation.

Layout strategy (per core, batch b fixed):
  xT[m]  : [c, n]  (c on partitions)  -- host pre-transposed, bf16
  qT/kT[m]: [d, n] = Wq[m].T-projection output, d on partitions
  v[m]   : [n_k, d] natural layout, with an extra per-head "ones" column so
           the PV matmul produces the softmax denominator Z in column 64.
  S^T    : [k, n] per (i,j,head) from lhsT=kT-slice, rhs=qT-slice (K=64)
  E = exp(S^T): ACT engine for most tasks; a tunable subset runs on DVE via
           a Schraudolph-style bit-trick (x*C+D -> int16 -> bitcast bf16),
           splitting the exp wall between both engines.
  PV     : out[n-sub, 65] with lhsT=E-slice, rhs=v-slice(65 cols) accumulated
           over key tiles; col 64 = Z * ones_val.
  consume: fused[n, c] += PV[:, 0:64] * reciprocal(Z*ones_val)  (DVE)
  final  : PE-transpose fused -> fusedT, out = fusedT.T @ Wp + bp, split in
           two contraction halves so most of it overlaps the attention tail.

Exact simplifications vs the reference:
  - bk drops entirely: q.(k+bk) adds a per-query constant across keys, which
    softmax over keys cancels.
  - bv folds into bp on the host: sum_k probs = 1, so the v-bias contributes
    (1/M) sum_ij mw_ij bv_j to every fused row; push through Wp into bp.
"""

import sys

import numpy as np

for _p in ("/opt/trn_rl_repo",):
    if _p not in sys.path:
        sys.path.insert(0, _p)

import ml_dtypes  # noqa: E402

import concourse.bass as bass  # noqa: E402
from concourse import bacc  # noqa: E402
import concourse.mybir as mybir  # noqa: E402
import concourse.tile as tile  # noqa: E402

M, B, N, C, H = 3, 8, 512, 512, 8
HD = C // H  # 64
P = 128
CT = C // P  # 4 contraction tiles
NT = N // P  # 4 row tiles
DT = C // P  # 4 output-channel tiles
SCALE = float(HD) ** -0.5

BF16 = mybir.dt.bfloat16
F32 = mybir.dt.float32
I16 = mybir.dt.int16
NP_BF16 = ml_dtypes.bfloat16

AluOp = mybir.AluOpType
ActFn = mybir.ActivationFunctionType

# Schraudolph bf16 exp: exp(x) ~= bitcast_bf16(int16(x*SCH_C + SCH_D))
SCH_SHIFT = 0.0579
SCH_C = 128.0 / float(np.log(2.0))
SCH_D = (127.0 - SCH_SHIFT) * 128.0 + 0.5  # +0.5: assume trunc-to-int store

# tasks whose exp runs on DVE instead of ACT (balance the two engines).
# Consume runs on Pool now, so DVE's only other load is the proj copies
# (early) — weight DVE-exp toward the mid/late stream. Never adjacent:
# each DVE task is emitted paired with its successor so ACT keeps lookahead.
DVE_EXP_TASKS = frozenset(range(10, 68, 3))


def _build_bass(mw, uniform, ones_val, reps=1, skip=()):
    """Emit the single-core SPMD program. mw is the [M,M] modal weight matrix
    (values are baked into the program as immediates)."""
    from concourse.masks import make_identity

    nc = bacc.Bacc(None)

    xt_d = nc.dram_tensor("xt", [M, P, CT, N], BF16, kind="ExternalInput")
    wq_d = nc.dram_tensor("wq", [M, P, CT, C], BF16, kind="ExternalInput")
    wk_d = nc.dram_tensor("wk", [M, P, CT, C], BF16, kind="ExternalInput")
    wv_d = nc.dram_tensor("wv", [M, P, CT, C], BF16, kind="ExternalInput")
    wp_d = nc.dram_tensor("wp", [P, CT, C], BF16, kind="ExternalInput")
    bq_d = nc.dram_tensor("bq", [P, M, DT], F32, kind="ExternalInput")
    bp_d = nc.dram_tensor("bp", [1, C], BF16, kind="ExternalInput")
    out_d = nc.dram_tensor("out", [N, C], F32, kind="ExternalOutput")

    with tile.TileContext(nc) as tc:
        with (
            tc.tile_pool(name="consts", bufs=1) as consts,
            tc.tile_pool(name="esb", bufs=3) as esb,
            tc.tile_pool(name="zr", bufs=8) as zrp,
            tc.tile_pool(name="spsum", bufs=3, space="PSUM") as spsum,
            tc.tile_pool(name="pvpsum", bufs=2, space="PSUM") as pvpsum,
        ):
            # ---- persistent SBUF tiles ----
            xt_sb = [consts.tile([P, CT, N], BF16, tag=f"xt{m}", name=f"xt{m}") for m in range(M)]
            wq_sb = [consts.tile([P, CT, C], BF16, tag=f"wq{m}", name=f"wq{m}") for m in range(M)]
            wk_sb = [consts.tile([P, CT, C], BF16, tag=f"wk{m}", name=f"wk{m}") for m in range(M)]
            wv_sb = [consts.tile([P, CT, C], BF16, tag=f"wv{m}", name=f"wv{m}") for m in range(M)]
            wp_sb = consts.tile([P, CT, C], BF16, tag="wp", name="wp")
            bq_sb = consts.tile([P, M, DT], F32, tag="bq", name="bq")
            bp_sb = consts.tile([1, C], BF16, tag="bp", name="bp")
            qT_sb = [consts.tile([P, DT, N], BF16, tag=f"qT{m}", name=f"qT{m}") for m in range(M)]
            kT_sb = [consts.tile([P, DT, N], BF16, tag=f"kT{m}", name=f"kT{m}") for m in range(M)]
            # v with per-head ones column (col 64) for the softmax denominator
            v_sb = [consts.tile([P, NT, H, HD + 1], BF16, tag=f"v{m}", name=f"v{m}") for m in range(M)]
            fused_sb = consts.tile([P, NT, C], F32, tag="fused", name="fused")
            fusedT_sb = consts.tile([P, CT, N], BF16, tag="fusedT", name="fusedT")
            out_sb = consts.tile([P, NT, C], F32, tag="outsb", name="outsb")
            ones_sb = consts.tile([1, P], BF16, tag="ones", name="ones")
            ident_sb = consts.tile([P, P], F32, tag="ident", name="ident")

            # ---- input DMAs: whole-tensor transfers (HWDGE dispatch is
            # ~625ns each and serial per queue, so spread dispatch across the
            # SP, ACT and DVE queues; m=0 first, DMA-complete sem adds ~900ns)
            nc.sync.dma_start(out=bq_sb, in_=bq_d[:])
            nc.sync.dma_start(out=xt_sb[0][:, 0:2, :], in_=xt_d[0][:, 0:2, :])
            nc.sync.dma_start(out=wq_sb[0][:, 0:2, :], in_=wq_d[0][:, 0:2, :])
            nc.sync.dma_start(out=xt_sb[0][:, 2:4, :], in_=xt_d[0][:, 2:4, :])
            nc.sync.dma_start(out=wq_sb[0][:, 2:4, :], in_=wq_d[0][:, 2:4, :])
            nc.sync.dma_start(out=wk_sb[0], in_=wk_d[0])
            nc.sync.dma_start(out=wv_sb[0], in_=wv_d[0])
            for m in (1, 2):
                nc.sync.dma_start(out=xt_sb[m], in_=xt_d[m])
                nc.sync.dma_start(out=wk_sb[m], in_=wk_d[m])
                nc.sync.dma_start(out=wv_sb[m], in_=wv_d[m])
                nc.sync.dma_start(out=wq_sb[m], in_=wq_d[m])
            nc.sync.dma_start(out=wp_sb, in_=wp_d[:])
            nc.sync.dma_start(out=bp_sb, in_=bp_d[:])

            nc.gpsimd.memset(ones_sb, 1.0)
            make_identity(nc, ident_sb)
            act_warm = consts.tile([1, 2], F32, tag="actwarm", name="actwarm")
            nc.scalar.activation(act_warm, ones_sb[0:1, 0:2], ActFn.Exp)

            # PE p-state warm-up: dummy matmuls with no DMA deps keep the
            # tensor engine continuously busy (ramping to max clock) while
            # the first input DMAs land.
            warm_sb = consts.tile([P, N], BF16, tag="warm", name="warm")
            nc.vector.memset(warm_sb, 0.0)
            wps = spsum.tile([P, N], F32, tag="s", name="s")
            for _ in range(8):
                nc.tensor.matmul(wps, warm_sb[:, 0:P], warm_sb, start=True, stop=True)

            def emit_qk_proj_dt(m, dt):
                # q (bias via DVE) and k (no bias: softmax-invariant) for one dt
                ps = spsum.tile([P, N], F32, tag="s", name="s")
                for ct in range(CT):
                    nc.tensor.matmul(
                        ps,
                        wq_sb[m][:, ct, dt * P : (dt + 1) * P],
                        xt_sb[m][:, ct, :],
                        start=(ct == 0),
                        stop=(ct == CT - 1),
                    )
                nc.vector.tensor_tensor(
                    qT_sb[m][:, dt, :],
                    ps,
                    bq_sb[:, m, dt : dt + 1].to_broadcast((P, N)),
                    AluOp.add,
                )
                ps = spsum.tile([P, N], F32, tag="s", name="s")
                for ct in range(CT):
                    nc.tensor.matmul(
                        ps,
                        wk_sb[m][:, ct, dt * P : (dt + 1) * P],
                        xt_sb[m][:, ct, :],
                        start=(ct == 0),
                        stop=(ct == CT - 1),
                    )
                nc.vector.tensor_copy(out=kT_sb[m][:, dt, :], in_=ps)

            def emit_v_proj(m, nts):
                # v projection -> natural layout [n_k, d]; bias folded into bp
                for nt in nts:
                    ps = spsum.tile([P, C], F32, tag="s", name="s")
                    for ct in range(CT):
                        nc.tensor.matmul(
                            ps,
                            xt_sb[m][:, ct, nt * P : (nt + 1) * P],
                            wv_sb[m][:, ct, :],
                            start=(ct == 0),
                            stop=(ct == CT - 1),
                        )
                    nc.vector.tensor_copy(
                        out=v_sb[m][:, nt, :, 0:HD],
                        in_=ps.rearrange("p (h e) -> p h e", e=HD),
                    )
                    nc.gpsimd.memset(v_sb[m][:, nt, :, HD : HD + 1], ones_val)

            def emit_qk_chunk(i, j, h, half, e_t, use_dve):
                hof = (h % 2) * HD
                ht = h // 2
                st = spsum.tile([P, 2, N], F32, tag="s", name="s")
                for k2 in range(2):
                    kt = half * 2 + k2
                    nc.tensor.matmul(
                        st[:, k2, :],
                        kT_sb[j][hof : hof + HD, ht, kt * P : (kt + 1) * P],
                        qT_sb[i][hof : hof + HD, ht, :],
                        start=True,
                        stop=True,
                    )
                dst = e_t[:, half * 2 : half * 2 + 2, :]
                if use_dve:
                    nc.vector.tensor_scalar(
                        dst.bitcast(I16),
                        st,
                        SCH_C,
                        SCH_D,
                        AluOp.mult,
                        AluOp.add,
                    )
                else:
                    nc.scalar.activation(dst, st, ActFn.Exp)

            def emit_qk_exp(i, j, h, use_dve):
                e_t = esb.tile([P, CT, N], BF16, tag="E", name="E")
                for half in range(2):
                    emit_qk_chunk(i, j, h, half, e_t, use_dve)
                return e_t

            def emit_pv_consume(i, j, h, e_t, first):
                w_ij = float(mw[i, j])
                pv_t = pvpsum.tile([P, NT, HD + 1], F32, tag="pv", name="pv")
                for ns in range(NT):
                    for kt in range(CT):
                        nc.tensor.matmul(
                            pv_t[:, ns, :],
                            e_t[:, kt, ns * P : (ns + 1) * P],
                            v_sb[j][:, kt, h, :],
                            start=(kt == 0),
                            stop=(kt == CT - 1),
                        )
                # consume: tmp = pv / (Z * ones_val) on DVE (Pool cannot read
                # PSUM), then fused += tmp on Pool (SBUF-only). The per-pair
                # modal weight is baked into ones_val on the uniform path.
                dst = fused_sb[:, :, h * HD : (h + 1) * HD]
                zr = zrp.tile([P, NT, 1], F32, tag="zr", name="zr")
                nc.vector.reciprocal(zr, pv_t[:, :, HD : HD + 1])
                if not uniform:
                    nc.vector.tensor_scalar(
                        zr, zr, w_ij / float(M), None, AluOp.mult
                    )
                if first:
                    nc.vector.tensor_tensor(
                        dst,
                        pv_t[:, :, 0:HD],
                        zr.to_broadcast((P, NT, HD)),
                        AluOp.mult,
                    )
                else:
                    tmp = zrp.tile([P, NT, HD], F32, tag="ctmp", name="ctmp")
                    nc.vector.tensor_tensor(
                        tmp,
                        pv_t[:, :, 0:HD],
                        zr.to_broadcast((P, NT, HD)),
                        AluOp.mult,
                    )
                    nc.gpsimd.tensor_tensor(dst, dst, tmp, AluOp.add)

            def emit_transpose_nt(ct, nt):
                tpf = spsum.tile([P, C], F32, tag="s", name="s")
                tp = tpf[:, 0:P]
                nc.tensor.transpose(
                    tp, fused_sb[:, nt, ct * P : (ct + 1) * P], ident_sb
                )
                dst = fusedT_sb[:, ct, nt * P : (nt + 1) * P]
                if ct == CT - 1:
                    # endgame: ACT is idle by now, DVE owns the consume tail
                    nc.scalar.copy(out=dst, in_=tp)
                else:
                    nc.vector.tensor_copy(out=dst, in_=tp)

            def emit_final_a_nt(nt):
                # first contraction half (ct 0,1) + bias; overlaps attention
                ps = spsum.tile([P, C], F32, tag="s", name="s")
                for ct in (0, 1):
                    nc.tensor.matmul(
                        ps,
                        fusedT_sb[:, ct, nt * P : (nt + 1) * P],
                        wp_sb[:, ct, :],
                        start=(ct == 0),
                        stop=False,
                    )
                nc.tensor.matmul(
                    ps, ones_sb[0:1, :], bp_sb[0:1, :], start=False, stop=True
                )
                nc.vector.tensor_copy(out=out_sb[:, nt, :], in_=ps)

            def emit_final_b_nt(nt):
                ps = spsum.tile([P, C], F32, tag="s", name="s")
                for ct in (2, 3):
                    nc.tensor.matmul(
                        ps,
                        fusedT_sb[:, ct, nt * P : (nt + 1) * P],
                        wp_sb[:, ct, :],
                        start=(ct == 2),
                        stop=(ct == 3),
                    )
                nc.vector.tensor_tensor(
                    out_sb[:, nt, :], out_sb[:, nt, :], ps, AluOp.add
                )
                nc.sync.dma_start(
                    out=out_d[nt * P : (nt + 1) * P, :], in_=out_sb[:, nt, :]
                )

            # ---- emission schedule ----
            # (0,0) runs all 8 heads first; the rest go head-major so fused
            # head-pair slices complete early and transpose mid-stream.
            rest_pairs = (
                (0, 1), (1, 0), (1, 1), (0, 2), (2, 0), (1, 2), (2, 1), (2, 2),
            )
            tasks = [(0, 0, h) for h in range(H)] + [
                (i, j, h) for h in range(H) for (i, j) in rest_pairs
            ]

            # proj chunks popped one per task emission; order chosen so every
            # task's qT/kT/v dependencies are emitted before first use.
            def chunk_list():
                return [
                    lambda: emit_qk_proj_dt(0, 1),
                    lambda: emit_v_proj(0, (0, 1)),
                    lambda: emit_v_proj(0, (2, 3)),
                    lambda: emit_qk_proj_dt(0, 2),
                    lambda: emit_qk_proj_dt(0, 3),
                    lambda: emit_qk_proj_dt(1, 0),
                    lambda: emit_v_proj(1, (0, 1)),
                    lambda: emit_v_proj(1, (2, 3)),
                    lambda: emit_qk_proj_dt(2, 0),
                    lambda: emit_v_proj(2, (0, 1)),
                    lambda: emit_v_proj(2, (2, 3)),
                    lambda: emit_qk_proj_dt(1, 1),
                    lambda: emit_qk_proj_dt(2, 1),
                    lambda: emit_qk_proj_dt(1, 2),
                    lambda: emit_qk_proj_dt(2, 2),
                    lambda: emit_qk_proj_dt(1, 3),
                    lambda: emit_qk_proj_dt(2, 3),
                ]
            # m/dt available after popping chunk index:
            qk_avail = {(0, 0): -1, (0, 1): 0, (0, 2): 3, (0, 3): 4,
                        (1, 0): 5, (2, 0): 8, (1, 1): 11, (2, 1): 12,
                        (1, 2): 13, (2, 2): 14, (1, 3): 15, (2, 3): 16}
            v_avail = {0: 2, 1: 7, 2: 10}

            for _rep in range(reps):
                emit_qk_proj_dt(0, 0)
                chunks = chunk_list()
                popped = -1  # index of last popped chunk
                consumed = [0] * H
                transposed = [False] * CT
                final_a_done = False
                pending = []
                deferred = []  # staggered transpose/final_a pieces

                def flush_one():
                    nonlocal final_a_done
                    pi, pj, ph, pe = pending.pop(0)
                    emit_pv_consume(pi, pj, ph, pe, first=(pi == 0 and pj == 0))
                    consumed[ph] += 1
                    ct = ph // 2
                    if (
                        not transposed[ct]
                        and consumed[2 * ct] == M * M
                        and consumed[2 * ct + 1] == M * M
                    ):
                        transposed[ct] = True
                        for nt in range(NT):
                            if ct == CT - 1:
                                # endgame: chain each ct3 transpose with its
                                # final_b piece so the tail interleaves
                                def _tail(nt=nt):
                                    emit_transpose_nt(CT - 1, nt)
                                    emit_final_b_nt(nt)

                                deferred.append(_tail)
                            else:
                                deferred.append(
                                    lambda ct=ct, nt=nt: emit_transpose_nt(ct, nt)
                                )
                        if not final_a_done and transposed[0] and transposed[1]:
                            final_a_done = True
                            for nt in range(NT):
                                deferred.append(
                                    lambda nt=nt: emit_final_a_nt(nt)
                                )
                    if deferred:
                        deferred.pop(0)()

                n = 0
                while n < len(tasks):
                    # a DVE-exp task is emitted together with its successor so
                    # ACT gets its next chunk without a full task-latency gap
                    group = [n]
                    if n in DVE_EXP_TASKS and n + 1 < len(tasks):
                        group.append(n + 1)
                    for g in group:
                        i, j, h = tasks[g]
                        assert qk_avail[(i, h // 2)] <= popped, (g, i, j, h)
                        assert qk_avail[(j, h // 2)] <= popped, (g, i, j, h)
                    if len(group) == 2:
                        # chunk-interleaved: ACT task (n+1) chunks lead so the
                        # ACT engine never waits a full task latency
                        nD, nA = group
                        eD = esb.tile([P, CT, N], BF16, tag="E", name="E")
                        eA = esb.tile([P, CT, N], BF16, tag="E", name="E")
                        for half in range(2):
                            iA, jA, hA = tasks[nA]
                            emit_qk_chunk(iA, jA, hA, half, eA, use_dve=False)
                            iD, jD, hD = tasks[nD]
                            emit_qk_chunk(iD, jD, hD, half, eD, use_dve=True)
                        pending.append((*tasks[nD], eD))
                        pending.append((*tasks[nA], eA))
                    else:
                        i, j, h = tasks[n]
                        e_t = emit_qk_exp(i, j, h, use_dve=False)
                        pending.append((i, j, h, e_t))
                    # one proj chunk per group (two back-to-back would starve
                    # ACT for ~3.4us of PE time)
                    if popped + 1 < len(chunks):
                        popped += 1
                        chunks[popped]()
                    while len(pending) > 1 and v_avail[pending[0][1]] <= popped:
                        flush_one()
                    assert len(pending) <= 3, n  # esb pool depth
                    n = group[-1] + 1
                while pending:
                    assert v_avail[pending[0][1]] <= popped
                    flush_one()
                assert all(transposed) and final_a_done
                while deferred:
                    deferred.pop(0)()

    nc.compile()
    return nc


def _prep_inputs(x, Wq, bq, Wk, bk, Wv, bv, mw, Wp, bp):
    """Host-side shard + retile. Returns (in_maps, uniform, ones_val, mw)."""
    x = np.asarray(x, dtype=np.float32)
    Wq = np.asarray(Wq, dtype=np.float32)
    bq = np.asarray(bq, dtype=np.float32)
    Wk = np.asarray(Wk, dtype=np.float32)
    Wv = np.asarray(Wv, dtype=np.float32)
    bv = np.asarray(bv, dtype=np.float64)
    mw = np.asarray(mw, dtype=np.float64)
    Wp = np.asarray(Wp, dtype=np.float64)
    bp = np.asarray(bp, dtype=np.float64)

    w0 = float(mw.flat[0])
    uniform = bool(np.all(mw == w0)) and abs(w0) > 1e-6
    if uniform:
        ones_val = float(NP_BF16(M / w0))
        # compensate bf16 rounding of ones_val exactly through Wp
        kappa = ones_val * w0 / M
    else:
        ones_val = 1.0
        kappa = 1.0

    # fold v-bias through Wp into bp: sum_k probs = 1 exactly, so each pair
    # contributes (mw_ij/M) * bv_j to every fused row.
    bias_v = (mw[:, :, None] * bv[None, :, :]).sum(axis=(0, 1)) / M  # [C]
    bp_eff = bp + bias_v @ Wp

    def tile_w(w):  # [C, C] -> [P, CT, C]
        return np.ascontiguousarray(
            w.reshape(CT, P, C).transpose(1, 0, 2).astype(NP_BF16)
        )

    wq_h = np.stack([tile_w(Wq[m] * SCALE) for m in range(M)])
    wk_h = np.stack([tile_w(Wk[m]) for m in range(M)])
    wv_h = np.stack([tile_w(Wv[m]) for m in range(M)])
    wp_h = tile_w((Wp / kappa).astype(np.float32))

    def tile_b(b):  # [C] -> [P, DT]
        return np.ascontiguousarray(b.reshape(DT, P).T.astype(np.float32))

    bq_h = np.ascontiguousarray(
        np.stack([tile_b(bq[m] * SCALE) for m in range(M)]).transpose(1, 0, 2)
    )
    bp_h = np.ascontiguousarray(bp_eff.reshape(1, C).astype(NP_BF16))

    # x [M,B,N,C] -> per-core xT [M,P,CT,N]
    xt_all = np.ascontiguousarray(
        x.transpose(1, 0, 3, 2)  # [B, M, C, N]
        .reshape(B, M, CT, P, N)
        .transpose(0, 1, 3, 2, 4)  # [B, M, P, CT, N]
        .astype(NP_BF16)
    )

    common = {
        "wq": wq_h,
        "wk": wk_h,
        "wv": wv_h,
        "wp": wp_h,
        "bq": bq_h,
        "bp": bp_h,
    }
    in_maps = [dict(common, xt=np.ascontiguousarray(xt_all[b])) for b in range(B)]
    return in_maps, uniform, ones_val, mw


def run(trace=False, **inputs):
    from concourse.bass_utils import run_bass_kernel_spmd

    in_maps, uniform, ones_val, mw = _prep_inputs(**inputs)
    nc = _build_bass(mw, uniform, ones_val)
    res = run_bass_kernel_spmd(
        nc, in_maps, core_ids=list(range(B)), trace=trace
    )
    out = np.stack([res.results[b]["out"] for b in range(B)]).astype(np.float32)
    return out, res


def kernel(**inputs):
    out, _ = run(trace=False, **inputs)
    return out


if __name__ == "__main__":
    rng = np.random.default_rng(0)
    ins = {
        "x": rng.standard_normal((M, B, N, C), dtype=np.float32),
        "Wq": rng.standard_normal((M, C, C), dtype=np.float32) * 0.02,
        "bq": rng.standard_normal((M, C), dtype=np.float32) * 0.02,
        "Wk": rng.standard_normal((M, C, C), dtype=np.float32) * 0.02,
        "bk": rng.standard_normal((M, C), dtype=np.float32) * 0.02,
        "Wv": rng.standard_normal((M, C, C), dtype=np.float32) * 0.02,
        "bv": rng.standard_normal((M, C), dtype=np.float32) * 0.02,
        "mw": np.ones((M, M), dtype=np.float32),
        "Wp": rng.standard_normal((C, C), dtype=np.float32) * 0.02,
        "bp": rng.standard_normal((C,), dtype=np.float32) * 0.02,
    }
    out = kernel(**ins)
    print("out", out.shape, out.dtype, float(np.abs(out).mean()))

